# revision 1
# baseline (speedup 1.0000x reference)
"""HaplotypeEmbedding Trainium2 kernel (self-contained).

Math: out = gelu(concat_l(tables[l][tok_l] * (tok_l != 0)) @ W1 + b1) @ W2 + b2

Device algorithm (per core, data-parallel over the N=131072 rows):
  The first matmul is algebraically folded into the embedding gather:
      x @ W1 = sum_l tables[l][tok_l] @ W1[l*256:(l+1)*256]
  so each core builds fused tables T[l] = tables[l] @ W1_l + b1/8 (fp16,
  on PE, written to a DRAM scratch), then per 128-row chunk:
    - dma_gather 8 fused rows/row (1KB fp16 rows, 4 SWDGE queues)
    - DVE pair-adds + PE identity-matmul accumulate -> PSUM fp32
    - ScalarE Gelu (erf) -> h fp16
    - PE transpose h -> hT, mm2 hT @ W2 -> PSUM, +b2 -> out f32
Row 0 of each fused table is b1/8 (tables row 0 zeroed), which reproduces
the reference's padding masking exactly.
"""
import numpy as np

import concourse.bass as bass
import concourse.tile as tile
import concourse.mybir as mybir
from concourse import bacc
from concourse.bass_utils import run_bass_kernel_spmd

F16 = mybir.dt.float16
F32 = mybir.dt.float32
I16 = mybir.dt.int16

L, V, D = 8, 512, 256
HID = 2 * D
B, K = 8192, 16
N = B * K
NCORES = 8
NPC = N // NCORES            # 16384 rows per core
NI = 2048                    # idxs per gather = 2 chunks of 128 rows
NGATH = NPC * L // NI        # 64 gathers per core
NCHUNK = NPC // 128          # 128 chunks
ACT_GELU = mybir.ActivationFunctionType.Gelu


def build_nc(npc=NPC, reps=1, act=ACT_GELU, queue_plan=None):
    gather_names = []
    ngath = npc * L // NI
    nc = bacc.Bacc("TRN2", target_bir_lowering=False, num_swdge_queues=4)
    tT = nc.dram_tensor("tablesT", [L, D, V], F32, kind="ExternalInput")
    w1 = nc.dram_tensor("W1", [L * D, HID], F32, kind="ExternalInput")
    w2 = nc.dram_tensor("W2", [HID, D], F32, kind="ExternalInput")
    b1 = nc.dram_tensor("b1", [1, HID], F32, kind="ExternalInput")
    b2 = nc.dram_tensor("b2", [1, D], F32, kind="ExternalInput")
    idx = nc.dram_tensor("idx", [128, npc * L // 16], I16, kind="ExternalInput")
    ident = nc.dram_tensor("ident", [128, 128], F16, kind="ExternalInput")
    outd = nc.dram_tensor("out", [npc, D], F32, kind="ExternalOutput")
    Td = nc.dram_tensor("Tscratch", [L * V, HID], F16, kind="Internal")

    with tile.TileContext(nc) as tc:
        with tc.tile_pool(name="const", bufs=1) as cpool:
            idxs = cpool.tile([128, npc * L // 16], I16)
            nc.sync.dma_start(idxs[:], idx[:])
            identt = cpool.tile([128, 128], F16)
            nc.sync.dma_start(identt[:], ident[:])
            w2f = cpool.tile([128, 4, D], F32)
            nc.sync.dma_start(w2f[:], w2.rearrange("(c p) n -> p c n", p=128))
            w2t = cpool.tile([128, 4, D], F16)
            nc.vector.tensor_copy(w2t[:], w2f[:])
            b1f = cpool.tile([1, HID], F32)
            nc.sync.dma_start(b1f[:], b1[:])
            b1row = cpool.tile([1, HID], F16)
            nc.vector.tensor_copy(b1row[:], b1f[:])
            b2f = cpool.tile([1, D], F32)
            nc.sync.dma_start(b2f[:], b2[:])
            b2row = cpool.tile([1, D], F16)
            nc.vector.tensor_copy(b2row[:], b2f[:])
            ones8 = cpool.tile([1, 128], F16)
            nc.gpsimd.memset(ones8[:], 0.125)
            ones1 = cpool.tile([1, 128], F16)
            nc.gpsimd.memset(ones1[:], 1.0)
            b1o8 = cpool.tile([128, HID], F32)
            b2t = cpool.tile([128, D], F32)

            # ---- setup: bias broadcast tiles + fused tables ----
            with (
                tc.tile_pool(name="setup", bufs=2) as spool,
                tc.tile_pool(name="spsum", bufs=2,
                             space=bass.MemorySpace.PSUM) as spsum,
            ):
                pb = spsum.tile([128, HID], F32, tag="pb")
                nc.tensor.matmul(pb[:], ones8[:], b1row[:], start=True, stop=True)
                nc.vector.tensor_copy(b1o8[:], pb[:])
                pb2 = spsum.tile([128, D], F32, tag="pb")
                nc.tensor.matmul(pb2[:], ones1[:], b2row[:], start=True, stop=True)
                nc.vector.tensor_copy(b2t[:], pb2[:])

                for l in range(L):
                    ttf = spool.tile([128, 2, V], F32, tag="ttf")
                    nc.sync.dma_start(
                        ttf[:], tT[l].rearrange("(dc p) v -> p dc v", p=128))
                    tt = spool.tile([128, 2, V], F16, tag="tt")
                    nc.vector.tensor_copy(tt[:], ttf[:])
                    nc.gpsimd.memset(tt[:, :, 0:1], 0.0)  # padding row insurance
                    w1f = spool.tile([128, 2, HID], F32, tag="w1f")
                    nc.sync.dma_start(
                        w1f[:], w1[l * D:(l + 1) * D].rearrange(
                            "(dc p) h -> p dc h", p=128))
                    w1t = spool.tile([128, 2, HID], F16, tag="w1")
                    nc.vector.tensor_copy(w1t[:], w1f[:])
                    for v4 in range(4):
                        pT = spsum.tile([128, HID], F32, tag="pT")
                        for dc in range(2):
                            nc.tensor.matmul(
                                pT[:], tt[:, dc, v4 * 128:(v4 + 1) * 128],
                                w1t[:, dc, :], start=(dc == 0), stop=(dc == 1))
                        ts = spool.tile([128, HID], F16, tag="ts")
                        nc.vector.tensor_add(ts[:], pT[:], b1o8[:])
                        nc.sync.dma_start(
                            Td[(l * 4 + v4) * 128:(l * 4 + v4 + 1) * 128, :], ts[:])

            # ---- main loop ----
            with (
                tc.tile_pool(name="g", bufs=4) as gpool,
                tc.tile_pool(name="work", bufs=8) as wpool,
                tc.tile_pool(name="hh", bufs=3) as hpool,
                tc.tile_pool(name="ob", bufs=3) as opool,
                tc.tile_pool(name="ph", bufs=2,
                             space=bass.MemorySpace.PSUM) as phpool,
                tc.tile_pool(name="pt", bufs=2,
                             space=bass.MemorySpace.PSUM) as ptpool,
                tc.tile_pool(name="po", bufs=2,
                             space=bass.MemorySpace.PSUM) as popool,
            ):
                def body():
                    for g in range(ngath):
                        gt = gpool.tile([128, 16, HID], F16, tag="g")
                        qn = 0 if queue_plan is None else queue_plan[len(gather_names) % ngath]
                        gi = nc.gpsimd.dma_gather(
                            gt[:], Td[:],
                            idxs[:, g * (NI // 16):(g + 1) * (NI // 16)],
                            NI, NI, HID,
                            transpose=False, single_packet=False,
                            queue_num=qn)
                        gather_names.append(gi.ins.name)
                        for ch in range(2):
                            # pair-adds on DVE: 8 -> 4
                            pairs = []
                            for q in range(4):
                                pq = wpool.tile([128, HID], F16, tag="pair")
                                nc.vector.tensor_add(
                                    pq[:], gt[:, ch * 8 + 2 * q, :],
                                    gt[:, ch * 8 + 2 * q + 1, :])
                                pairs.append(pq)
                            # PE identity-accumulate 4 -> PSUM f32
                            ph = phpool.tile([128, HID], F32, tag="ph")
                            for q in range(4):
                                nc.tensor.matmul(
                                    ph[:], identt[:], pairs[q][:],
                                    start=(q == 0), stop=(q == 3))
                            # Gelu -> h fp16 in SBUF
                            h = hpool.tile([128, HID], F16, tag="h")
                            nc.scalar.activation(h[:], ph[:], act)
                            # PE transpose h -> hT (psum f16), copy to SBUF
                            pt = ptpool.tile([128, 4, 128], F16, tag="pt")
                            for c in range(4):
                                nc.tensor.transpose(
                                    pt[:, c, :], h[:, c * 128:(c + 1) * 128],
                                    identt[:])
                            ht = hpool.tile([128, 4, 128], F16, tag="ht")
                            nc.vector.tensor_copy(ht[:], pt[:])
                            # mm2: out = h @ W2 (+ b2)
                            po = popool.tile([128, D], F32, tag="po")
                            for c in range(4):
                                nc.tensor.matmul(
                                    po[:], ht[:, c, :], w2t[:, c, :],
                                    start=(c == 0), stop=(c == 3))
                            ob = opool.tile([128, D], F32, tag="ob")
                            nc.vector.tensor_add(ob[:], po[:], b2t[:])
                            r0 = (2 * g + ch) * 128
                            nc.sync.dma_start(outd[r0:r0 + 128, :], ob[:])

                if reps == 1:
                    body()
                else:
                    with tc.For_i(0, reps, 1):
                        body()
    nc.compile()
    return nc, gather_names


def _gather_lanes(nc, gather_names):
    from concourse.tile_scheduler import PROC_NAME_TO_IDX
    base = PROC_NAME_TO_IDX["DMASW0"]
    lanes = {}
    for i, name in enumerate(gather_names):
        inst = nc.inst_map[name]
        lanes[i] = inst.bass_scheduled_proc - base
    return lanes


def build_nc_tuned(npc=NPC, reps=1, act=ACT_GELU):
    nc1, names1 = build_nc(npc, reps, act)
    lanes = _gather_lanes(nc1, names1)
    ngath = npc * L // NI
    plan = [lanes[g] % 4 for g in range(ngath)]
    nc2, names2 = build_nc(npc, reps, act, queue_plan=plan)
    lanes2 = _gather_lanes(nc2, names2)
    for g in range(ngath):
        assert lanes2[g] % 4 == plan[g % ngath], (g, lanes2[g], plan[g])
    return nc2


def _host_inputs(haplotypes, tables, W1, b1, W2, b2, npc=NPC):
    tok = np.clip(np.asarray(haplotypes).reshape(N, L), 0, V - 1).astype(np.int16)
    tablesT = np.ascontiguousarray(
        np.asarray(tables, dtype=np.float32).transpose(0, 2, 1))
    common = {
        "tablesT": tablesT,
        "W1": np.asarray(W1, dtype=np.float32),
        "W2": np.asarray(W2, dtype=np.float32),
        "b1": np.asarray(b1, dtype=np.float32).reshape(1, HID),
        "b2": np.asarray(b2, dtype=np.float32).reshape(1, D),
        "ident": np.eye(128, dtype=np.float16),
    }
    loff = (np.arange(L, dtype=np.int16) * V)
    in_maps = []
    for c in range(NCORES):
        tc_ = tok[c * npc:(c + 1) * npc]                      # [npc, 8]
        v = tc_.reshape(npc // 256, 2, 128, L).transpose(0, 1, 3, 2) \
            + loff[None, None, :, None]
        w = v.reshape(npc * L // NI, 128, 16).transpose(2, 0, 1) \
            .reshape(16, npc * L // 16)
        in_maps.append({**common, "idx": np.tile(w, (8, 1))})
    return in_maps


_NC_CACHE = {}


def kernel(haplotypes, tables, W1, b1, W2, b2):
    if "nc" not in _NC_CACHE:
        _NC_CACHE["nc"] = build_nc_tuned()
    nc = _NC_CACHE["nc"]
    in_maps = _host_inputs(haplotypes, tables, W1, b1, W2, b2)
    res = run_bass_kernel_spmd(nc, in_maps, core_ids=list(range(NCORES)))
    out = np.concatenate([res.results[c]["out"] for c in range(NCORES)], axis=0)
    return out.reshape(B, K, D).astype(np.float32)

